# revision 9
# baseline (speedup 1.0000x reference)
"""Trainium2 Bass kernel for nn_GAT_7507602833557 (8-core SPMD GAT), v2.

Sharding: query-node rows split across 8 cores (512 rows each); keys/values
replicated. Per-core adjacency slice is passed pre-transposed ([keys, own
queries]) in bf16 ({0,1} values are exact in bf16).

Math (per attention map, 9 maps: 6 spatial + 2 intent + 1 output):
  e[i,j] = leakyrelu(f1[i] + f2[j], 0.2); softmax over masked j; att @ V.
  exp(lrelu(f1+f2)) = e^{f1[i]} * C[j] * max(P[j], Q[i]) with
  P=e^{0.8 f2}, C=e^{0.2 f2}, Q=e^{-0.8 f1}; the e^{f1[i]} factor cancels
  in the softmax. C is folded into the value matrix (whp), so the map is
  m[j,i] = max(P[j], Q[i]) * adj[j,i]: one tensor_scalar(max) + one
  tensor_tensor(mult) per (head, key-tile) on DVE, or one
  scalar_tensor_tensor on gpsimd.

v2 structure vs v1: one merged build matmul per key tile produces Wh for
all 8 heads + the P/C scores (augmented [97,280] weight matrix: x rows,
ones row, intent rows); attention matmuls are value-stationary
(LDW 33 cols, stream 512 moving bf16) accumulating feature-major
[33*heads, 512] PSUM banks; epilogue transposes back to query-major.
"""
import os
import numpy as np

import concourse.bass as bass
import concourse.bacc as bacc
import concourse.tile as tile
from concourse import mybir
from concourse.bass_utils import run_bass_kernel_spmd
from concourse.masks import make_identity

import ml_dtypes

N, NIN, NHID, NOUT = 4096, 64, 32, 64
NHEADS, D_INT = 8, 32
H_SP, H_INT = 6, 2
NCORES = 8
R = N // NCORES           # 512 own query rows per core
JT = N // 128             # 32 key tiles
IT = R // 128             # 4 own query tiles
XR = NIN + D_INT + 1      # 97 augmented input rows (x, intent, ones)
WC = NHEADS * 33          # 264 value cols (per head: den + 32 feats)
F32 = mybir.dt.float32
BF16 = mybir.dt.bfloat16
# every STRIDE-th l1 map op-pair runs as one STT on gpsimd
STRIDE = int(os.environ.get("KERNEL_STRIDE", "6"))
STRIDE2 = int(os.environ.get("KERNEL_STRIDE2", "8"))
WORK_BUFS = int(os.environ.get("KERNEL_WORK_BUFS", "6"))
# heads -> (bank, slot): 4 banks x 2 heads; PSUM matmul outputs need
# 0/32/64-aligned base partitions, so slots sit at partition 0 and 64
NBANK = 4
HEAD_ORDER = [6, 7, 0, 1, 2, 3, 4, 5]


def _build_program():
    nc = bacc.Bacc("TRN2", target_bir_lowering=False, debug=False,
                   num_devices=NCORES)
    d_xia = nc.dram_tensor("xia", [XR, N], F32, kind="ExternalInput")
    d_xio = nc.dram_tensor("xio", [XR, R], F32, kind="ExternalInput")
    d_adjT = nc.dram_tensor("adjT", [N, R], BF16, kind="ExternalInput")
    d_wsp = nc.dram_tensor("wsp", [H_SP, NIN, NHID], F32, kind="ExternalInput")
    d_asp = nc.dram_tensor("asp", [H_SP, 2 * NHID], F32, kind="ExternalInput")
    d_wint = nc.dram_tensor("wint", [H_INT, NIN, NHID], F32, kind="ExternalInput")
    d_aint = nc.dram_tensor("aint", [H_INT, 2 * D_INT], F32, kind="ExternalInput")
    d_wout = nc.dram_tensor("wout", [NHEADS * NHID, NOUT], F32, kind="ExternalInput")
    d_aout = nc.dram_tensor("aout", [2 * NOUT], F32, kind="ExternalOutput"
                            if False else "ExternalInput")
    d_out = nc.dram_tensor("out", [R, NOUT], F32, kind="ExternalOutput")

    with tile.TileContext(nc) as tc:
        _kernel_body(tc, d_xia, d_xio, d_adjT, d_wsp, d_asp, d_wint, d_aint,
                     d_wout, d_aout, d_out)
    nc.compile()
    return nc


def _kernel_body(tc, d_xia, d_xio, d_adjT, d_wsp, d_asp, d_wint, d_aint,
                 d_wout, d_aout, d_out):
    nc = tc.nc
    Act = mybir.ActivationFunctionType
    Alu = mybir.AluOpType

    from contextlib import ExitStack
    ctx = ExitStack()
    big = ctx.enter_context(tc.tile_pool(name="big", bufs=1))
    work = ctx.enter_context(tc.tile_pool(name="work", bufs=WORK_BUFS))
    ps = ctx.enter_context(tc.tile_pool(name="ps", bufs=2, space="PSUM"))
    psb = ctx.enter_context(tc.tile_pool(name="psb", bufs=2, space="PSUM"))
    pacc = ctx.enter_context(tc.tile_pool(name="pacc", bufs=1, space="PSUM"))
    ext = ctx.enter_context(tc.tile_pool(name="ext", bufs=2))
    dram = ctx.enter_context(tc.tile_pool(name="dram", bufs=1, space="DRAM"))

    # ---------------- loads (critical-path-first order) --------------------
    apair = big.tile([NHID, 2 * H_SP], F32, tag="apair")
    nc.sync.dma_start(out=apair, in_=d_asp.ap().rearrange("h (c o) -> o (h c)", c=2))
    aintp = big.tile([D_INT, 2 * H_INT], F32, tag="aintp")
    nc.sync.dma_start(out=aintp, in_=d_aint.ap().rearrange("h (c o) -> o (h c)", c=2))
    xio = big.tile([XR, R], F32, tag="xio")
    nc.sync.dma_start(out=xio, in_=d_xio.ap())
    adjT_sb = big.tile([128, JT, R], BF16, tag="adjT_sb")

    def load_adj(g):
        nc.sync.dma_start(
            out=adjT_sb[:, 4 * g:4 * (g + 1), :],
            in_=d_adjT.ap()[4 * g * 128:4 * (g + 1) * 128, :]
                .rearrange("(t p) i -> p t i", p=128))
    load_adj(0)

    # w_aug [97, 280]: value cols 0:264 (per head: den-ones col + 32 W cols),
    # P cols 264:272, C cols 272:280
    w_aug = big.tile([XR, WC + 16], F32, tag="w_aug")
    nc.vector.memset(w_aug, 0.0)
    w_aug_h = w_aug[:, 0:WC].rearrange("f (h c) -> f h c", c=33)
    nc.sync.dma_start(out=w_aug_h[0:NIN, 0:H_SP, 1:33],
                      in_=d_wsp.ap().rearrange("h f o -> f h o"))
    nc.sync.dma_start(out=w_aug_h[0:NIN, H_SP:NHEADS, 1:33],
                      in_=d_wint.ap().rearrange("h f o -> f h o"))
    xia = big.tile([XR, N], F32, tag="xia")
    for g in range(2):
        nc.sync.dma_start(out=xia[:, 2048 * g:2048 * (g + 1)],
                          in_=d_xia.ap()[:, 2048 * g:2048 * (g + 1)])
    for g in range(1, 8):
        load_adj(g)
    wout_f = big.tile([128, 2, NOUT], F32, tag="wout_f")
    nc.sync.dma_start(out=wout_f, in_=d_wout.ap().rearrange("(c p) o -> p c o", p=128))
    aout_sb = big.tile([NOUT, 2], F32, tag="aout_sb")
    nc.sync.dma_start(out=aout_sb, in_=d_aout.ap().rearrange("(c o) -> o c", c=2))

    # den-ones entries: w_aug[96, 33h] = 1
    nc.gpsimd.memset(w_aug_h[96:97, :, 0:1], 1.0)

    ident = big.tile([128, 128], F32, tag="ident")
    make_identity(nc, ident)
    ident_b = big.tile([128, 128], BF16, tag="ident_b")
    make_identity(nc, ident_b)

    def tr(out, in_, idt):
        p = in_.partition_size()
        nc.tensor.transpose(out, in_, idt[0:p, 0:p])

    # ---------------- spatial a-vectors pre-projected through W ------------
    # wt [32, 6, 64] = W_h^T
    wt = big.tile([NHID, H_SP, NIN], F32, tag="wt")
    for grp in range(2):
        ptw = ps.tile([NHID, 3 * NIN], F32, tag="ps")
        for k in range(3):
            h = 3 * grp + k
            tr(ptw[:, NIN * k:NIN * (k + 1)], w_aug_h[0:NIN, h, 1:33], ident)
        nc.scalar.copy(out=wt[:, 3 * grp:3 * grp + 3, :], in_=ptw)
    # pw [64, 12]: cols (2h, 2h+1) = (W_h@a1_h, W_h@a2_h)
    pw = ps.tile([NIN, 2 * H_SP], F32, tag="ps")
    for h in range(H_SP):
        nc.tensor.matmul(pw[:, 2 * h:2 * h + 2], wt[:, h, :],
                         apair[:, 2 * h:2 * h + 2])
    pw_hc = pw.rearrange("f (h c) -> f c h", c=2)
    # score cols of w_aug: P = 0.8*w2 / 0.8*a2int, C = 0.2*...
    nc.scalar.mul(out=w_aug[0:NIN, WC + 0:WC + H_SP], in_=pw_hc[:, 1, :], mul=0.8)
    nc.scalar.mul(out=w_aug[0:NIN, WC + 8:WC + 8 + H_SP], in_=pw_hc[:, 1, :], mul=0.2)
    ai_hc = aintp[:].rearrange("f (h c) -> f c h", c=2)
    nc.scalar.mul(out=w_aug[NIN:NIN + D_INT, WC + H_SP:WC + 8], in_=ai_hc[:, 1, :], mul=0.8)
    nc.scalar.mul(out=w_aug[NIN:NIN + D_INT, WC + 8 + H_SP:WC + 16], in_=ai_hc[:, 1, :], mul=0.2)
    # wqa [97, 8]: -0.8 * a1 projections for the Q rows
    wqa = big.tile([XR, NHEADS], F32, tag="wqa")
    nc.vector.memset(wqa, 0.0)
    nc.scalar.mul(out=wqa[0:NIN, 0:H_SP], in_=pw_hc[:, 0, :], mul=-0.8)
    nc.scalar.mul(out=wqa[NIN:NIN + D_INT, H_SP:NHEADS], in_=ai_hc[:, 0, :], mul=-0.8)

    # ---------------- Q rows + broadcast -----------------------------------
    pq = ps.tile([NHEADS, R], F32, tag="ps")
    nc.tensor.matmul(pq, wqa, xio)
    qrow = big.tile([NHEADS, R], BF16, tag="qrow")
    nc.scalar.activation(out=qrow, in_=pq, func=Act.Exp)
    qrd = dram.tile([NHEADS, R], BF16, tag="qrd")
    nc.sync.dma_start(out=qrd, in_=qrow)
    qb = big.tile([128, NHEADS, R], BF16, tag="qb")
    for h in HEAD_ORDER:
        nc.gpsimd.dma_start(out=qb[:, h, :],
                            in_=qrd[h:h + 1, :].to_broadcast([128, R]))
    ones1 = big.tile([1, 128], F32, tag="ones1")
    nc.vector.memset(ones1, 1.0)

    # ---------------- build loop: whp + P/C scores per key tile ------------
    etP = big.tile([128, JT, 8], F32, tag="etP")
    whp = big.tile([128, JT, NHEADS, 33], BF16, tag="whp")
    for jt in range(JT):
        pbld = psb.tile([128, WC + 16], F32, tag="pbld")
        nc.tensor.matmul(pbld, xia[:, 128 * jt:128 * (jt + 1)], w_aug)
        nc.scalar.activation(out=etP[:, jt, :], in_=pbld[:, WC:WC + 8],
                             func=Act.Exp)
        etC = ext.tile([128, 8], BF16, tag="etC")
        nc.scalar.activation(out=etC, in_=pbld[:, WC + 8:WC + 16], func=Act.Exp)
        nc.vector.tensor_tensor(
            whp[:, jt, :, :],
            pbld[:, 0:WC].rearrange("p (h c) -> p h c", c=33),
            etC[:].to_broadcast([128, 8, 33]), Alu.mult)

    # ---------------- layer 1 attention ------------------------------------
    accs = [pacc.tile([97, R], F32, tag=f"acc{b}",
                      name=f"acc_l1_{b}") for b in range(NBANK)]
    k = 0
    for h in HEAD_ORDER:
        b, slot = h // 2, h % 2
        out_sl = accs[b][64 * slot:64 * slot + 33, :]
        for jc in range(JT):
            t = work.tile([128, R], BF16, tag="t")
            nc.vector.tensor_scalar(out=t, in0=qb[:, h, :],
                                    scalar1=etP[:, jc, h:h + 1],
                                    scalar2=None, op0=Alu.max)
            m = work.tile([128, R], BF16, tag="m")
            eng = (nc.gpsimd if STRIDE and k % STRIDE == 0 else nc.vector)
            eng.tensor_tensor(m, t, adjT_sb[:, jc, :], Alu.mult)
            k += 1
            nc.tensor.matmul(out_sl, whp[:, jc, h, :], m,
                             start=(jc == 0), stop=(jc == JT - 1))

    # ---------------- l1 epilogue: transpose, normalize, elu ---------------
    accs_sb = []
    for b in range(NBANK):
        t_sb = big.tile([97, R], F32, tag=f"accsb{b}")
        nc.scalar.copy(out=t_sb, in_=accs[b])
        accs_sb.append(t_sb)
    h_nat = big.tile([128, IT, NHEADS * NHID], BF16, tag="h_nat")
    for it in range(IT):
        psq = ps.tile([128, NBANK * 97], F32, tag="ps")
        for b in range(NBANK):
            tr(psq[:, 97 * b:97 * (b + 1)],
               accs_sb[b][:, 128 * it:128 * (it + 1)], ident)
        psq_b = psq.rearrange("p (b c) -> p b c", c=97)
        rec = ext.tile([128, NHEADS], F32, tag="rec")
        rec_h = rec.rearrange("p (b s) -> p b s", s=2)
        nc.vector.reciprocal(out=rec_h[:, :, 0], in_=psq_b[:, :, 0])
        nc.vector.reciprocal(out=rec_h[:, :, 1], in_=psq_b[:, :, 64])
        v = ext.tile([128, NHEADS * NHID], BF16, tag="v")
        vv = v.rearrange("p (h o) -> p h o", h=NHEADS)
        for h in range(NHEADS):
            nc.vector.tensor_scalar(
                out=vv[:, h, :],
                in0=psq_b[:, h // 2, 64 * (h % 2) + 1:64 * (h % 2) + 33],
                scalar1=rec_h[:, h // 2, h % 2:h % 2 + 1], scalar2=None,
                op0=Alu.mult)
        e = ext.tile([128, NHEADS * NHID], BF16, tag="e")
        nc.scalar.activation(out=e, in_=v, func=Act.Exp)
        em1 = ext.tile([128, NHEADS * NHID], BF16, tag="em1")
        nc.vector.tensor_scalar(out=em1, in0=e, scalar1=-1.0, scalar2=None,
                                op0=Alu.add)
        r = ext.tile([128, NHEADS * NHID], BF16, tag="r")
        nc.vector.tensor_scalar(out=r, in0=v, scalar1=0.0, scalar2=None,
                                op0=Alu.max)
        nc.vector.tensor_tensor(h_nat[:, it, :], em1, r, Alu.min)

    # ---------------- output layer: Who, o1/o2, payload --------------------
    hT = big.tile([128, 2, R], BF16, tag="hT")
    for fc in range(2):
        ph = ps.tile([128, R], BF16, tag="ps")
        for it in range(IT):
            tr(ph[:, 128 * it:128 * (it + 1)],
               h_nat[:, it, 128 * fc:128 * (fc + 1)], ident_b)
        nc.scalar.copy(out=hT[:, fc, :], in_=ph)
    wout_m = big.tile([128, 2, NOUT], BF16, tag="wout_m")
    nc.scalar.copy(out=wout_m, in_=wout_f)
    pwho = ps.tile([NOUT, R], F32, tag="ps")
    for fc in range(2):
        nc.tensor.matmul(pwho, wout_m[:, fc, :], hT[:, fc, :],
                         start=(fc == 0), stop=(fc == 1))
    whoT_f = big.tile([NOUT, R], F32, tag="whoT_f")
    nc.scalar.copy(out=whoT_f, in_=pwho)
    whoT_b = big.tile([NOUT, R], BF16, tag="whoT_b")
    nc.scalar.copy(out=whoT_b, in_=pwho)
    po1 = ps.tile([1, R], F32, tag="ps")
    nc.tensor.matmul(po1, aout_sb[:, 0:1], whoT_f)
    po2 = ps.tile([1, R], F32, tag="ps")
    nc.tensor.matmul(po2, aout_sb[:, 1:2], whoT_f)
    # Qo row = exp(-0.8 o1) broadcast down partitions via K=1 matmul
    qo_row = big.tile([1, R], F32, tag="qo_row")
    nc.scalar.activation(out=qo_row, in_=po1, func=Act.Exp, scale=-0.8)
    o2_row = big.tile([1, R], F32, tag="o2_row")
    nc.scalar.copy(out=o2_row, in_=po2)
    pqob = ps.tile([128, R], F32, tag="ps")
    nc.tensor.matmul(pqob, ones1, qo_row)
    qob = big.tile([128, R], BF16, tag="qob")
    nc.scalar.copy(out=qob, in_=pqob)
    # o2 transposed for payload P/C columns
    po2T = ps.tile([128, IT], F32, tag="ps")
    for it in range(IT):
        tr(po2T[:, it:it + 1], o2_row[:, 128 * it:128 * (it + 1)], ident)
    coT = big.tile([128, IT], F32, tag="coT")
    nc.scalar.activation(out=coT, in_=po2T, func=Act.Exp, scale=0.2)
    poT = big.tile([128, IT], F32, tag="poT")
    nc.scalar.activation(out=poT, in_=po2T, func=Act.Exp, scale=0.8)
    # payload [R, 66] built transposed: cols 0:64 Co*Who, 64 Co, 65 Po
    payT = big.tile([128, IT, NOUT + 2], BF16, tag="payT")
    for it in range(IT):
        ppt = ps.tile([128, NOUT], BF16, tag="ps")
        tr(ppt, whoT_b[:, 128 * it:128 * (it + 1)], ident_b)
        nc.scalar.mul(out=payT[:, it, 0:NOUT], in_=ppt, mul=coT[:, it:it + 1])
        nc.scalar.copy(out=payT[:, it, NOUT:NOUT + 1], in_=coT[:, it:it + 1])
        nc.scalar.copy(out=payT[:, it, NOUT + 1:NOUT + 2], in_=poT[:, it:it + 1])
    ccin = dram.tile([R, NOUT + 2], BF16, tag="ccin")
    ccout = dram.tile([N, NOUT + 2], BF16, tag="ccout")
    nc.sync.dma_start(out=ccin.rearrange("(k p) c -> p k c", p=128), in_=payT)
    if os.environ.get("KERNEL_SIMCC"):
        for d in range(NCORES):
            nc.sync.dma_start(out=ccout[R * d:R * (d + 1), :], in_=ccin)
    else:
        nc.gpsimd.collective_compute(
            "AllGather", mybir.AluOpType.bypass,
            replica_groups=[list(range(NCORES))],
            ins=[ccin.opt()], outs=[ccout.opt()])
    whop = big.tile([128, JT, NOUT + 2], BF16, tag="whop")
    nc.sync.dma_start(out=whop, in_=ccout.rearrange("(t p) c -> p t c", p=128))
    poT2 = big.tile([128, JT], F32, tag="poT2")
    nc.scalar.copy(out=poT2, in_=whop[:, :, NOUT + 1])

    # ---------------- output attention -------------------------------------
    acc2 = pacc.tile([NOUT + 1, R], F32, tag="acc0", name="acc_l2")
    for jc in range(JT):
        t2 = work.tile([128, R], BF16, tag="t")
        nc.vector.tensor_scalar(out=t2, in0=qob,
                                scalar1=poT2[:, jc:jc + 1],
                                scalar2=None, op0=Alu.max)
        m2 = work.tile([128, R], BF16, tag="m")
        eng2 = (nc.gpsimd if STRIDE2 and jc % STRIDE2 == 0 else nc.vector)
        eng2.tensor_tensor(m2, t2, adjT_sb[:, jc, :], Alu.mult)
        nc.tensor.matmul(acc2, whop[:, jc, 0:NOUT + 1], m2,
                         start=(jc == 0), stop=(jc == JT - 1))

    # ---------------- out = tanh(num/den) ----------------------------------
    acc2_sb = big.tile([NOUT + 1, R], F32, tag="acc2_sb")
    nc.scalar.copy(out=acc2_sb, in_=acc2)
    out_sb = big.tile([128, IT, NOUT], F32, tag="out_sb")
    for it in range(IT):
        psq2 = ps.tile([128, NOUT + 1], F32, tag="ps")
        tr(psq2, acc2_sb[:, 128 * it:128 * (it + 1)], ident)
        rec2 = ext.tile([128, 1], F32, tag="rec2")
        nc.vector.reciprocal(out=rec2, in_=psq2[:, NOUT:NOUT + 1])
        nc.scalar.activation(out=out_sb[:, it, :], in_=psq2[:, 0:NOUT],
                             func=Act.Tanh, scale=rec2)
    nc.sync.dma_start(out=d_out.ap().rearrange("(k p) c -> p k c", p=128),
                      in_=out_sb)
    ctx.close()


_NC_CACHE = None


def _get_nc():
    global _NC_CACHE
    if _NC_CACHE is None:
        _NC_CACHE = _build_program()
    return _NC_CACHE


def _make_in_maps(inputs):
    x = np.asarray(inputs["x"], np.float32)
    adj = np.asarray(inputs["adj"], np.float32)
    ie = np.asarray(inputs["intent_embeds"], np.float32)
    xia_full = np.concatenate(
        [x.T, ie.T, np.ones((1, N), np.float32)], axis=0)
    xia_full = np.ascontiguousarray(xia_full)
    in_maps = []
    for d in range(NCORES):
        sl = slice(d * R, (d + 1) * R)
        in_maps.append({
            "xia": xia_full,
            "xio": np.ascontiguousarray(xia_full[:, sl]),
            "adjT": np.ascontiguousarray(adj[sl, :].T).astype(ml_dtypes.bfloat16),
            "wsp": np.asarray(inputs["W_sp"], np.float32),
            "asp": np.asarray(inputs["a_sp"], np.float32),
            "wint": np.asarray(inputs["W_int"], np.float32),
            "aint": np.asarray(inputs["a_int"], np.float32),
            "wout": np.asarray(inputs["W_out"], np.float32),
            "aout": np.asarray(inputs["a_out"], np.float32),
        })
    return in_maps


def kernel(x, adj, intent_embeds, W_sp, a_sp, W_int, a_int, W_out, a_out):
    nc = _get_nc()
    in_maps = _make_in_maps(dict(
        x=x, adj=adj, intent_embeds=intent_embeds, W_sp=W_sp, a_sp=a_sp,
        W_int=W_int, a_int=a_int, W_out=W_out, a_out=a_out))
    res = run_bass_kernel_spmd(nc, in_maps, list(range(NCORES)))
    return np.concatenate([res.results[d]["out"] for d in range(NCORES)], axis=0)


# revision 12
# speedup vs baseline: 1.0276x; 1.0276x over previous
"""Trainium2 Bass kernel for nn_GAT_7507602833557 (8-core SPMD GAT), v2.

Sharding: query-node rows split across 8 cores (512 rows each); keys/values
replicated. Per-core adjacency slice is passed pre-transposed ([keys, own
queries]) in bf16 ({0,1} values are exact in bf16).

Math (per attention map, 9 maps: 6 spatial + 2 intent + 1 output):
  e[i,j] = leakyrelu(f1[i] + f2[j], 0.2); softmax over masked j; att @ V.
  exp(lrelu(f1+f2)) = e^{f1[i]} * C[j] * max(P[j], Q[i]) with
  P=e^{0.8 f2}, C=e^{0.2 f2}, Q=e^{-0.8 f1}; the e^{f1[i]} factor cancels
  in the softmax. C is folded into the value matrix (whp), so the map is
  m[j,i] = max(P[j], Q[i]) * adj[j,i]: one tensor_scalar(max) + one
  tensor_tensor(mult) per (head, key-tile) on DVE, or one
  scalar_tensor_tensor on gpsimd.

v2 structure vs v1: one merged build matmul per key tile produces Wh for
all 8 heads + the P/C scores (augmented [97,280] weight matrix: x rows,
ones row, intent rows); attention matmuls are value-stationary
(LDW 33 cols, stream 512 moving bf16) accumulating feature-major
[33*heads, 512] PSUM banks; epilogue transposes back to query-major.
"""
import os
import numpy as np

import concourse.bass as bass
import concourse.bacc as bacc
import concourse.tile as tile
from concourse import mybir
from concourse.bass_utils import run_bass_kernel_spmd
from concourse.masks import make_identity

import ml_dtypes

N, NIN, NHID, NOUT = 4096, 64, 32, 64
NHEADS, D_INT = 8, 32
H_SP, H_INT = 6, 2
NCORES = 8
R = N // NCORES           # 512 own query rows per core
JT = N // 128             # 32 key tiles
IT = R // 128             # 4 own query tiles
XR = NIN + D_INT + 1      # 97 augmented input rows (x, intent, ones)
WC = NHEADS * 33          # 264 value cols (per head: den + 32 feats)
F32 = mybir.dt.float32
BF16 = mybir.dt.bfloat16
# every STRIDE-th l1 map op-pair runs as one STT on gpsimd
STRIDE = int(os.environ.get("KERNEL_STRIDE", "6"))
STRIDE2 = int(os.environ.get("KERNEL_STRIDE2", "8"))
WORK_BUFS = int(os.environ.get("KERNEL_WORK_BUFS", "6"))
# heads -> (bank, slot): 4 banks x 2 heads; PSUM matmul outputs need
# 0/32/64-aligned base partitions, so slots sit at partition 0 and 64
NBANK = 4
HEAD_ORDER = [6, 7, 0, 1, 2, 3, 4, 5]


def _build_program():
    nc = bacc.Bacc("TRN2", target_bir_lowering=False, debug=False,
                   num_devices=NCORES)
    d_xia = nc.dram_tensor("xia", [XR, N], F32, kind="ExternalInput")
    d_xio = nc.dram_tensor("xio", [XR, R], F32, kind="ExternalInput")
    d_adjT = nc.dram_tensor("adjT", [N, R], BF16, kind="ExternalInput")
    d_wsp = nc.dram_tensor("wsp", [H_SP, NIN, NHID], F32, kind="ExternalInput")
    d_asp = nc.dram_tensor("asp", [H_SP, 2 * NHID], F32, kind="ExternalInput")
    d_wint = nc.dram_tensor("wint", [H_INT, NIN, NHID], F32, kind="ExternalInput")
    d_aint = nc.dram_tensor("aint", [H_INT, 2 * D_INT], F32, kind="ExternalInput")
    d_wout = nc.dram_tensor("wout", [NHEADS * NHID, NOUT], F32, kind="ExternalInput")
    d_aout = nc.dram_tensor("aout", [2 * NOUT], F32, kind="ExternalOutput"
                            if False else "ExternalInput")
    d_out = nc.dram_tensor("out", [R, NOUT], F32, kind="ExternalOutput")

    with tile.TileContext(nc) as tc:
        _kernel_body(tc, d_xia, d_xio, d_adjT, d_wsp, d_asp, d_wint, d_aint,
                     d_wout, d_aout, d_out)
    nc.compile()
    return nc


def _kernel_body(tc, d_xia, d_xio, d_adjT, d_wsp, d_asp, d_wint, d_aint,
                 d_wout, d_aout, d_out):
    nc = tc.nc
    Act = mybir.ActivationFunctionType
    Alu = mybir.AluOpType

    from contextlib import ExitStack
    ctx = ExitStack()
    big = ctx.enter_context(tc.tile_pool(name="big", bufs=1))
    work = ctx.enter_context(tc.tile_pool(name="work", bufs=WORK_BUFS))
    ps = ctx.enter_context(tc.tile_pool(name="ps", bufs=2, space="PSUM"))
    psb = ctx.enter_context(tc.tile_pool(name="psb", bufs=2, space="PSUM"))
    pacc = ctx.enter_context(tc.tile_pool(name="pacc", bufs=1, space="PSUM"))
    ext = ctx.enter_context(tc.tile_pool(name="ext", bufs=2))
    work8 = ctx.enter_context(tc.tile_pool(
        name="work8", bufs=int(os.environ.get("KERNEL_W8", "3"))))
    dram = ctx.enter_context(tc.tile_pool(name="dram", bufs=1, space="DRAM"))

    # ---------------- loads (critical-path-first order) --------------------
    apair = big.tile([NHID, 2 * H_SP], F32, tag="apair")
    nc.sync.dma_start(out=apair, in_=d_asp.ap().rearrange("h (c o) -> o (h c)", c=2))
    aintp = big.tile([D_INT, 2 * H_INT], F32, tag="aintp")
    nc.sync.dma_start(out=aintp, in_=d_aint.ap().rearrange("h (c o) -> o (h c)", c=2))
    xio = big.tile([XR, R], F32, tag="xio")
    nc.sync.dma_start(out=xio, in_=d_xio.ap())
    adjT_sb = big.tile([128, JT, R], BF16, tag="adjT_sb")

    def load_adj(g):
        nc.sync.dma_start(
            out=adjT_sb[:, 4 * g:4 * (g + 1), :],
            in_=d_adjT.ap()[4 * g * 128:4 * (g + 1) * 128, :]
                .rearrange("(t p) i -> p t i", p=128))
    load_adj(0)

    # w_aug [97, 280]: value cols 0:264 (per head: den-ones col + 32 W cols),
    # P cols 264:272, C cols 272:280
    w_aug = big.tile([XR, WC + 16], F32, tag="w_aug")
    nc.vector.memset(w_aug, 0.0)
    w_aug_h = w_aug[:, 0:WC].rearrange("f (h c) -> f h c", c=33)
    nc.scalar.dma_start(out=w_aug_h[0:NIN, 0:H_SP, 1:33],
                        in_=d_wsp.ap().rearrange("h f o -> f h o"))
    nc.scalar.dma_start(out=w_aug_h[0:NIN, H_SP:NHEADS, 1:33],
                        in_=d_wint.ap().rearrange("h f o -> f h o"))
    xia = big.tile([XR, N], F32, tag="xia")
    for g in range(1, 8):
        load_adj(g)
    wout_f = big.tile([128, 2, NOUT], F32, tag="wout_f")
    aout_sb = big.tile([NOUT, 2], F32, tag="aout_sb")

    # den-ones entries: w_aug[96, 33h] = 1
    nc.gpsimd.memset(w_aug_h[96:97, :, 0:1], 1.0)

    ident = big.tile([128, 128], F32, tag="ident")
    make_identity(nc, ident)
    ident_b = big.tile([128, 128], BF16, tag="ident_b")
    make_identity(nc, ident_b)

    def tr(out, in_, idt):
        p = in_.partition_size()
        nc.tensor.transpose(out, in_, idt[0:p, 0:p])

    # ---------------- spatial a-vectors pre-projected through W ------------
    # wt [32, 6, 64] = W_h^T
    wt = big.tile([NHID, H_SP, NIN], F32, tag="wt")
    for grp in range(2):
        ptw = ps.tile([NHID, 3 * NIN], F32, tag="ps")
        for k in range(3):
            h = 3 * grp + k
            tr(ptw[:, NIN * k:NIN * (k + 1)], w_aug_h[0:NIN, h, 1:33], ident)
        nc.scalar.copy(out=wt[:, 3 * grp:3 * grp + 3, :], in_=ptw)
    # pw [64, 12]: cols (2h, 2h+1) = (W_h@a1_h, W_h@a2_h)
    pw = ps.tile([NIN, 2 * H_SP], F32, tag="ps")
    for h in range(H_SP):
        nc.tensor.matmul(pw[:, 2 * h:2 * h + 2], wt[:, h, :],
                         apair[:, 2 * h:2 * h + 2])
    pw_hc = pw.rearrange("f (h c) -> f c h", c=2)
    # score cols of w_aug: P = 0.8*w2 / 0.8*a2int, C = 0.2*...
    nc.scalar.mul(out=w_aug[0:NIN, WC + 0:WC + H_SP], in_=pw_hc[:, 1, :], mul=0.8)
    nc.scalar.mul(out=w_aug[0:NIN, WC + 8:WC + 8 + H_SP], in_=pw_hc[:, 1, :], mul=0.2)
    ai_hc = aintp[:].rearrange("f (h c) -> f c h", c=2)
    nc.scalar.mul(out=w_aug[NIN:NIN + D_INT, WC + H_SP:WC + 8], in_=ai_hc[:, 1, :], mul=0.8)
    nc.scalar.mul(out=w_aug[NIN:NIN + D_INT, WC + 8 + H_SP:WC + 16], in_=ai_hc[:, 1, :], mul=0.2)
    # wqa [97, 8]: +0.8 * a1 projections (D-row: exp(0.8 f1))
    wqa = big.tile([XR, NHEADS], F32, tag="wqa")
    nc.vector.memset(wqa, 0.0)
    nc.scalar.mul(out=wqa[0:NIN, 0:H_SP], in_=pw_hc[:, 0, :], mul=0.8)
    nc.scalar.mul(out=wqa[NIN:NIN + D_INT, H_SP:NHEADS], in_=ai_hc[:, 0, :], mul=0.8)

    # ---------------- Q rows + broadcast -----------------------------------
    pq = ps.tile([NHEADS, R], F32, tag="ps")
    nc.tensor.matmul(pq, wqa, xio)
    qrow = big.tile([NHEADS, R], BF16, tag="qrow")
    nc.scalar.activation(out=qrow, in_=pq, func=Act.Exp)
    qrd = dram.tile([NHEADS, R], BF16, tag="qrd")
    nc.gpsimd.dma_start(out=qrd, in_=qrow)
    qb = big.tile([128, NHEADS, R], BF16, tag="qb")
    for h in HEAD_ORDER:
        nc.gpsimd.dma_start(out=qb[:, h, :],
                            in_=qrd[h:h + 1, :].to_broadcast([128, R]))
    nc.gpsimd.dma_start(out=wout_f, in_=d_wout.ap().rearrange("(c p) o -> p c o", p=128))
    nc.gpsimd.dma_start(out=aout_sb, in_=d_aout.ap().rearrange("(c o) -> o c", c=2))
    for g in range(4):
        nc.gpsimd.dma_start(out=xia[:, 1024 * g:1024 * (g + 1)],
                            in_=d_xia.ap()[:, 1024 * g:1024 * (g + 1)])
    ones1 = big.tile([1, 128], F32, tag="ones1")
    nc.vector.memset(ones1, 1.0)

    # ---------------- build loop: whp + P/C scores per key tile ------------
    etP = big.tile([128, JT, 8], F32, tag="etP")
    whp = big.tile([128, JT, NHEADS, 33], BF16, tag="whp")
    for jt in range(JT):
        pbld = psb.tile([128, WC + 16], F32, tag="pbld")
        nc.tensor.matmul(pbld, xia[:, 128 * jt:128 * (jt + 1)], w_aug)
        nc.scalar.activation(out=etP[:, jt, :], in_=pbld[:, WC:WC + 8],
                             func=Act.Exp)
        etC = ext.tile([128, 8], BF16, tag="etC")
        nc.scalar.activation(out=etC, in_=pbld[:, WC + 8:WC + 16], func=Act.Exp)
        nc.vector.tensor_tensor(
            whp[:, jt, :, :],
            pbld[:, 0:WC].rearrange("p (h c) -> p h c", c=33),
            etC[:].to_broadcast([128, 8, 33]), Alu.mult)

    # ---------------- layer 1 attention ------------------------------------
    # jc-outer: per key tile, all 8 heads' scores t8 (TS on DVE or relu on
    # ACT for assisted heads), one batched mask TT (adj broadcast across
    # heads) + gpsimd singles, then value-stationary MMs. ACT-assisted heads
    # compute t-1 via relu and add the +1 term with a second MM on raw adj.
    accs = [pacc.tile([97, R], F32, tag=f"acc{b}",
                      name=f"acc_l1_{b}") for b in range(NBANK)]
    PAT = os.environ.get("KERNEL_PAT", "AVAVAGAV")  # per (jc*8+h) pair
    GPH = int(os.environ.get("KERNEL_GPH", "2"))    # heads 8-GPH.. on gpsimd TT
    for jc in range(JT):
        t8 = work8.tile([128, NHEADS, R], BF16, tag="t8")
        kinds = []
        for h in range(NHEADS):
            kind = PAT[(jc * NHEADS + h) % len(PAT)]
            kinds.append(kind)
            if kind == "A":
                nc.scalar.activation(out=t8[:, h, :], in_=qb[:, h, :],
                                     func=Act.Relu,
                                     scale=etP[:, jc, h:h + 1], bias=-1.0)
            else:
                nc.vector.tensor_scalar(out=t8[:, h, :], in0=qb[:, h, :],
                                        scalar1=etP[:, jc, h:h + 1],
                                        scalar2=1.0, op0=Alu.mult, op1=Alu.max)
        m8 = work8.tile([128, NHEADS, R], BF16, tag="m8")
        nv = NHEADS - GPH
        aap = adjT_sb[:, jc, :]
        if nv > 0:
            bc = bass.AP(aap.tensor, aap.offset,
                         [list(aap.ap[0]), [0, nv], list(aap.ap[1])])
            nc.vector.tensor_tensor(m8[:, 0:nv, :], t8[:, 0:nv, :], bc, Alu.mult)
        for h in range(nv, NHEADS):
            nc.gpsimd.tensor_tensor(m8[:, h, :], t8[:, h, :], aap, Alu.mult)
        for h in range(NHEADS):
            b, slot = h // 2, h % 2
            out_sl = accs[b][64 * slot:64 * slot + 33, :]
            if kinds[h] == "A":
                nc.tensor.matmul(out_sl, whp[:, jc, h, :], aap,
                                 start=(jc == 0), stop=False)
                nc.tensor.matmul(out_sl, whp[:, jc, h, :], m8[:, h, :],
                                 start=False, stop=(jc == JT - 1))
            else:
                nc.tensor.matmul(out_sl, whp[:, jc, h, :], m8[:, h, :],
                                 start=(jc == 0), stop=(jc == JT - 1))

    # ---------------- l1 epilogue: transpose, normalize, elu ---------------
    accs_sb = []
    for b in range(NBANK):
        t_sb = big.tile([97, R], F32, tag=f"accsb{b}")
        nc.scalar.copy(out=t_sb, in_=accs[b])
        accs_sb.append(t_sb)
    h_nat = big.tile([128, IT, NHEADS * NHID], BF16, tag="h_nat")
    for it in range(IT):
        psq = ps.tile([128, NBANK * 97], F32, tag="ps")
        for b in range(NBANK):
            tr(psq[:, 97 * b:97 * (b + 1)],
               accs_sb[b][:, 128 * it:128 * (it + 1)], ident)
        psq_b = psq.rearrange("p (b c) -> p b c", c=97)
        rec = ext.tile([128, NHEADS], F32, tag="rec")
        rec_h = rec.rearrange("p (b s) -> p b s", s=2)
        nc.vector.reciprocal(out=rec_h[:, :, 0], in_=psq_b[:, :, 0])
        nc.vector.reciprocal(out=rec_h[:, :, 1], in_=psq_b[:, :, 64])
        v = ext.tile([128, NHEADS * NHID], BF16, tag="v")
        vv = v.rearrange("p (h o) -> p h o", h=NHEADS)
        for h in range(NHEADS):
            nc.vector.tensor_scalar(
                out=vv[:, h, :],
                in0=psq_b[:, h // 2, 64 * (h % 2) + 1:64 * (h % 2) + 33],
                scalar1=rec_h[:, h // 2, h % 2:h % 2 + 1], scalar2=None,
                op0=Alu.mult)
        e = ext.tile([128, NHEADS * NHID], BF16, tag="e")
        nc.scalar.activation(out=e, in_=v, func=Act.Exp)
        em1 = ext.tile([128, NHEADS * NHID], BF16, tag="em1")
        nc.vector.tensor_scalar(out=em1, in0=e, scalar1=-1.0, scalar2=None,
                                op0=Alu.add)
        r = ext.tile([128, NHEADS * NHID], BF16, tag="r")
        nc.vector.tensor_scalar(out=r, in0=v, scalar1=0.0, scalar2=None,
                                op0=Alu.max)
        nc.vector.tensor_tensor(h_nat[:, it, :], em1, r, Alu.min)

    # ---------------- output layer: Who, o1/o2, payload --------------------
    hT = big.tile([128, 2, R], BF16, tag="hT")
    for fc in range(2):
        ph = ps.tile([128, R], BF16, tag="ps")
        for it in range(IT):
            tr(ph[:, 128 * it:128 * (it + 1)],
               h_nat[:, it, 128 * fc:128 * (fc + 1)], ident_b)
        nc.scalar.copy(out=hT[:, fc, :], in_=ph)
    wout_m = big.tile([128, 2, NOUT], BF16, tag="wout_m")
    nc.scalar.copy(out=wout_m, in_=wout_f)
    pwho = ps.tile([NOUT, R], F32, tag="ps")
    for fc in range(2):
        nc.tensor.matmul(pwho, wout_m[:, fc, :], hT[:, fc, :],
                         start=(fc == 0), stop=(fc == 1))
    whoT_f = big.tile([NOUT, R], F32, tag="whoT_f")
    nc.scalar.copy(out=whoT_f, in_=pwho)
    whoT_b = big.tile([NOUT, R], BF16, tag="whoT_b")
    nc.scalar.copy(out=whoT_b, in_=pwho)
    po1 = ps.tile([1, R], F32, tag="ps")
    nc.tensor.matmul(po1, aout_sb[:, 0:1], whoT_f)
    po2 = ps.tile([1, R], F32, tag="ps")
    nc.tensor.matmul(po2, aout_sb[:, 1:2], whoT_f)
    # Qo row = exp(-0.8 o1) broadcast down partitions via K=1 matmul
    qo_row = big.tile([1, R], F32, tag="qo_row")
    nc.scalar.activation(out=qo_row, in_=po1, func=Act.Exp, scale=0.8)
    o2_row = big.tile([1, R], F32, tag="o2_row")
    nc.scalar.copy(out=o2_row, in_=po2)
    pqob = ps.tile([128, R], F32, tag="ps")
    nc.tensor.matmul(pqob, ones1, qo_row)
    qob = big.tile([128, R], BF16, tag="qob")
    nc.scalar.copy(out=qob, in_=pqob)
    # o2 transposed for payload P/C columns
    po2T = ps.tile([128, IT], F32, tag="ps")
    for it in range(IT):
        tr(po2T[:, it:it + 1], o2_row[:, 128 * it:128 * (it + 1)], ident)
    coT = big.tile([128, IT], F32, tag="coT")
    nc.scalar.activation(out=coT, in_=po2T, func=Act.Exp, scale=0.2)
    poT = big.tile([128, IT], F32, tag="poT")
    nc.scalar.activation(out=poT, in_=po2T, func=Act.Exp, scale=0.8)
    # payload [R, 66] built transposed: cols 0:64 Co*Who, 64 Co, 65 Po
    payT = big.tile([128, IT, NOUT + 2], BF16, tag="payT")
    for it in range(IT):
        ppt = ps.tile([128, NOUT], BF16, tag="ps")
        tr(ppt, whoT_b[:, 128 * it:128 * (it + 1)], ident_b)
        nc.scalar.mul(out=payT[:, it, 0:NOUT], in_=ppt, mul=coT[:, it:it + 1])
        nc.scalar.copy(out=payT[:, it, NOUT:NOUT + 1], in_=coT[:, it:it + 1])
        nc.scalar.copy(out=payT[:, it, NOUT + 1:NOUT + 2], in_=poT[:, it:it + 1])
    ccin = dram.tile([R, NOUT + 2], BF16, tag="ccin")
    ccout = dram.tile([N, NOUT + 2], BF16, tag="ccout")
    nc.sync.dma_start(out=ccin.rearrange("(k p) c -> p k c", p=128), in_=payT)
    if os.environ.get("KERNEL_SIMCC"):
        for d in range(NCORES):
            nc.sync.dma_start(out=ccout[R * d:R * (d + 1), :], in_=ccin)
    else:
        nc.gpsimd.collective_compute(
            "AllGather", mybir.AluOpType.bypass,
            replica_groups=[list(range(NCORES))],
            ins=[ccin.opt()], outs=[ccout.opt()])
    whop = big.tile([128, JT, NOUT + 2], BF16, tag="whop")
    nc.sync.dma_start(out=whop, in_=ccout.rearrange("(t p) c -> p t c", p=128))
    poT2 = big.tile([128, JT], F32, tag="poT2")
    nc.scalar.copy(out=poT2, in_=whop[:, :, NOUT + 1])

    # ---------------- output attention -------------------------------------
    acc2 = pacc.tile([NOUT + 1, R], F32, tag="acc0", name="acc_l2")
    PAT2 = os.environ.get("KERNEL_PAT2", "AVAV")
    for jg in range(JT // 4):
        t4 = work8.tile([128, 4, R], BF16, tag="t4")
        kinds2 = []
        for q in range(4):
            jc = 4 * jg + q
            kind = PAT2[jc % len(PAT2)]
            kinds2.append(kind)
            if kind == "A":
                nc.scalar.activation(out=t4[:, q, :], in_=qob,
                                     func=Act.Relu,
                                     scale=poT2[:, jc:jc + 1], bias=-1.0)
            else:
                nc.vector.tensor_scalar(out=t4[:, q, :], in0=qob,
                                        scalar1=poT2[:, jc:jc + 1],
                                        scalar2=1.0, op0=Alu.mult, op1=Alu.max)
        m4 = work8.tile([128, 4, R], BF16, tag="m4")
        nc.vector.tensor_tensor(m4, t4, adjT_sb[:, 4 * jg:4 * jg + 4, :],
                                Alu.mult)
        for q in range(4):
            jc = 4 * jg + q
            first = jc == 0
            last = jc == JT - 1
            if kinds2[q] == "A":
                nc.tensor.matmul(acc2, whop[:, jc, 0:NOUT + 1],
                                 adjT_sb[:, jc, :], start=first, stop=False)
                nc.tensor.matmul(acc2, whop[:, jc, 0:NOUT + 1], m4[:, q, :],
                                 start=False, stop=last)
            else:
                nc.tensor.matmul(acc2, whop[:, jc, 0:NOUT + 1], m4[:, q, :],
                                 start=first, stop=last)

    # ---------------- out = tanh(num/den) ----------------------------------
    acc2_sb = big.tile([NOUT + 1, R], F32, tag="acc2_sb")
    nc.scalar.copy(out=acc2_sb, in_=acc2)
    out_sb = big.tile([128, IT, NOUT], F32, tag="out_sb")
    for it in range(IT):
        psq2 = ps.tile([128, NOUT + 1], F32, tag="ps")
        tr(psq2, acc2_sb[:, 128 * it:128 * (it + 1)], ident)
        rec2 = ext.tile([128, 1], F32, tag="rec2")
        nc.vector.reciprocal(out=rec2, in_=psq2[:, NOUT:NOUT + 1])
        nc.scalar.activation(out=out_sb[:, it, :], in_=psq2[:, 0:NOUT],
                             func=Act.Tanh, scale=rec2)
    nc.sync.dma_start(out=d_out.ap().rearrange("(k p) c -> p k c", p=128),
                      in_=out_sb)
    ctx.close()


_NC_CACHE = None


def _get_nc():
    global _NC_CACHE
    if _NC_CACHE is None:
        _NC_CACHE = _build_program()
    return _NC_CACHE


def _make_in_maps(inputs):
    x = np.asarray(inputs["x"], np.float32)
    adj = np.asarray(inputs["adj"], np.float32)
    ie = np.asarray(inputs["intent_embeds"], np.float32)
    xia_full = np.concatenate(
        [x.T, ie.T, np.ones((1, N), np.float32)], axis=0)
    xia_full = np.ascontiguousarray(xia_full)
    in_maps = []
    for d in range(NCORES):
        sl = slice(d * R, (d + 1) * R)
        in_maps.append({
            "xia": xia_full,
            "xio": np.ascontiguousarray(xia_full[:, sl]),
            "adjT": np.ascontiguousarray(adj[sl, :].T).astype(ml_dtypes.bfloat16),
            "wsp": np.asarray(inputs["W_sp"], np.float32),
            "asp": np.asarray(inputs["a_sp"], np.float32),
            "wint": np.asarray(inputs["W_int"], np.float32),
            "aint": np.asarray(inputs["a_int"], np.float32),
            "wout": np.asarray(inputs["W_out"], np.float32),
            "aout": np.asarray(inputs["a_out"], np.float32),
        })
    return in_maps


def kernel(x, adj, intent_embeds, W_sp, a_sp, W_int, a_int, W_out, a_out):
    nc = _get_nc()
    in_maps = _make_in_maps(dict(
        x=x, adj=adj, intent_embeds=intent_embeds, W_sp=W_sp, a_sp=a_sp,
        W_int=W_int, a_int=a_int, W_out=W_out, a_out=a_out))
    res = run_bass_kernel_spmd(nc, in_maps, list(range(NCORES)))
    return np.concatenate([res.results[d]["out"] for d in range(NCORES)], axis=0)


# revision 16
# speedup vs baseline: 1.4024x; 1.3648x over previous
"""Trainium2 Bass kernel for nn_GAT_7507602833557 (8-core SPMD GAT), v2.

Sharding: query-node rows split across 8 cores (512 rows each); keys/values
replicated. Per-core adjacency slice is passed pre-transposed ([keys, own
queries]) in bf16 ({0,1} values are exact in bf16).

Math (per attention map, 9 maps: 6 spatial + 2 intent + 1 output):
  e[i,j] = leakyrelu(f1[i] + f2[j], 0.2); softmax over masked j; att @ V.
  exp(lrelu(f1+f2)) = e^{f1[i]} * C[j] * max(P[j], Q[i]) with
  P=e^{0.8 f2}, C=e^{0.2 f2}, Q=e^{-0.8 f1}; the e^{f1[i]} factor cancels
  in the softmax. C is folded into the value matrix (whp), so the map is
  m[j,i] = max(P[j], Q[i]) * adj[j,i]: one tensor_scalar(max) + one
  tensor_tensor(mult) per (head, key-tile) on DVE, or one
  scalar_tensor_tensor on gpsimd.

v2 structure vs v1: one merged build matmul per key tile produces Wh for
all 8 heads + the P/C scores (augmented [97,280] weight matrix: x rows,
ones row, intent rows); attention matmuls are value-stationary
(LDW 33 cols, stream 512 moving bf16) accumulating feature-major
[33*heads, 512] PSUM banks; epilogue transposes back to query-major.
"""
import os
import numpy as np

import concourse.bass as bass
import concourse.bacc as bacc
import concourse.tile as tile
from concourse import mybir
from concourse.bass_utils import run_bass_kernel_spmd
from concourse.masks import make_identity

import ml_dtypes

N, NIN, NHID, NOUT = 4096, 64, 32, 64
NHEADS, D_INT = 8, 32
H_SP, H_INT = 6, 2
NCORES = 8
R = N // NCORES           # 512 own query rows per core
JT = N // 128             # 32 key tiles
IT = R // 128             # 4 own query tiles
XR = NIN + D_INT + 1      # 97 augmented input rows (x, intent, ones)
WC = NHEADS * 33          # 264 value cols (per head: den + 32 feats)
F32 = mybir.dt.float32
BF16 = mybir.dt.bfloat16
# every STRIDE-th l1 map op-pair runs as one STT on gpsimd
STRIDE = int(os.environ.get("KERNEL_STRIDE", "6"))
STRIDE2 = int(os.environ.get("KERNEL_STRIDE2", "8"))
WORK_BUFS = int(os.environ.get("KERNEL_WORK_BUFS", "6"))
# heads -> (bank, slot): 4 banks x 2 heads; PSUM matmul outputs need
# 0/32/64-aligned base partitions, so slots sit at partition 0 and 64
NBANK = 4
HEAD_ORDER = [6, 7, 0, 1, 2, 3, 4, 5]


def _build_program():
    nc = bacc.Bacc("TRN2", target_bir_lowering=False, debug=False,
                   num_devices=NCORES)
    d_xia = nc.dram_tensor("xia", [XR, N], F32, kind="ExternalInput")
    d_xio = nc.dram_tensor("xio", [XR, R], F32, kind="ExternalInput")
    d_adjT = nc.dram_tensor("adjT", [N, R], BF16, kind="ExternalInput")
    d_wsp = nc.dram_tensor("wsp", [H_SP, NIN, NHID], F32, kind="ExternalInput")
    d_asp = nc.dram_tensor("asp", [H_SP, 2 * NHID], F32, kind="ExternalInput")
    d_wint = nc.dram_tensor("wint", [H_INT, NIN, NHID], F32, kind="ExternalInput")
    d_aint = nc.dram_tensor("aint", [H_INT, 2 * D_INT], F32, kind="ExternalInput")
    d_wout = nc.dram_tensor("wout", [NHEADS * NHID, NOUT], F32, kind="ExternalInput")
    d_aout = nc.dram_tensor("aout", [2 * NOUT], F32, kind="ExternalOutput"
                            if False else "ExternalInput")
    d_out = nc.dram_tensor("out", [R, NOUT], F32, kind="ExternalOutput")

    with tile.TileContext(nc) as tc:
        _kernel_body(tc, d_xia, d_xio, d_adjT, d_wsp, d_asp, d_wint, d_aint,
                     d_wout, d_aout, d_out)
    nc.compile()
    return nc


def _kernel_body(tc, d_xia, d_xio, d_adjT, d_wsp, d_asp, d_wint, d_aint,
                 d_wout, d_aout, d_out):
    nc = tc.nc
    Act = mybir.ActivationFunctionType
    Alu = mybir.AluOpType

    from contextlib import ExitStack
    ctx = ExitStack()
    big = ctx.enter_context(tc.tile_pool(name="big", bufs=1))
    work = ctx.enter_context(tc.tile_pool(name="work", bufs=WORK_BUFS))
    ps = ctx.enter_context(tc.tile_pool(name="ps", bufs=2, space="PSUM"))
    psb = ctx.enter_context(tc.tile_pool(name="psb", bufs=2, space="PSUM"))
    pacc = ctx.enter_context(tc.tile_pool(name="pacc", bufs=1, space="PSUM"))
    ext = ctx.enter_context(tc.tile_pool(name="ext", bufs=2))
    work8 = ctx.enter_context(tc.tile_pool(
        name="work8", bufs=int(os.environ.get("KERNEL_W8", "3"))))
    dram = ctx.enter_context(tc.tile_pool(name="dram", bufs=1, space="DRAM"))

    # ---------------- loads (critical-path-first order) --------------------
    apair = big.tile([NHID, 2 * H_SP], F32, tag="apair")
    nc.sync.dma_start(out=apair, in_=d_asp.ap().rearrange("h (c o) -> o (h c)", c=2))
    aintp = big.tile([D_INT, 2 * H_INT], F32, tag="aintp")
    nc.sync.dma_start(out=aintp, in_=d_aint.ap().rearrange("h (c o) -> o (h c)", c=2))
    xio = big.tile([XR, R], F32, tag="xio")
    nc.sync.dma_start(out=xio, in_=d_xio.ap())
    adjT_sb = big.tile([128, JT, R], BF16, tag="adjT_sb")

    def load_adj(g):
        nc.sync.dma_start(
            out=adjT_sb[:, 4 * g:4 * (g + 1), :],
            in_=d_adjT.ap()[4 * g * 128:4 * (g + 1) * 128, :]
                .rearrange("(t p) i -> p t i", p=128))
    load_adj(0)

    # w_aug [97, 280]: value cols 0:264 (per head: den-ones col + 32 W cols),
    # P cols 264:272, C cols 272:280
    w_aug = big.tile([XR, WC + 16], F32, tag="w_aug")
    nc.vector.memset(w_aug, 0.0)
    w_aug_h = w_aug[:, 0:WC].rearrange("f (h c) -> f h c", c=33)
    nc.scalar.dma_start(out=w_aug_h[0:NIN, 0:H_SP, 1:33],
                        in_=d_wsp.ap().rearrange("h f o -> f h o"))
    nc.scalar.dma_start(out=w_aug_h[0:NIN, H_SP:NHEADS, 1:33],
                        in_=d_wint.ap().rearrange("h f o -> f h o"))
    xia = big.tile([XR, N], F32, tag="xia")
    for g in range(1, 8):
        load_adj(g)
    wout_f = big.tile([128, 2, NOUT], F32, tag="wout_f")
    aout_sb = big.tile([NOUT, 2], F32, tag="aout_sb")

    # den-ones entries: w_aug[96, 33h] = 1
    nc.gpsimd.memset(w_aug_h[96:97, :, 0:1], 1.0)

    ident = big.tile([128, 128], F32, tag="ident")
    make_identity(nc, ident)
    ident_b = big.tile([128, 128], BF16, tag="ident_b")
    make_identity(nc, ident_b)

    def tr(out, in_, idt):
        p = in_.partition_size()
        nc.tensor.transpose(out, in_, idt[0:p, 0:p])

    # ---------------- spatial a-vectors pre-projected through W ------------
    # wt [32, 6, 64] = W_h^T
    wt = big.tile([NHID, H_SP, NIN], F32, tag="wt")
    for grp in range(2):
        ptw = ps.tile([NHID, 3 * NIN], F32, tag="ps")
        for k in range(3):
            h = 3 * grp + k
            tr(ptw[:, NIN * k:NIN * (k + 1)], w_aug_h[0:NIN, h, 1:33], ident)
        nc.scalar.copy(out=wt[:, 3 * grp:3 * grp + 3, :], in_=ptw)
    # pw [64, 12]: cols (2h, 2h+1) = (W_h@a1_h, W_h@a2_h)
    pw = ps.tile([NIN, 2 * H_SP], F32, tag="ps")
    for h in range(H_SP):
        nc.tensor.matmul(pw[:, 2 * h:2 * h + 2], wt[:, h, :],
                         apair[:, 2 * h:2 * h + 2])
    pw_hc = pw.rearrange("f (h c) -> f c h", c=2)
    # score cols of w_aug: P = 0.8*w2 / 0.8*a2int, C = 0.2*...
    nc.scalar.mul(out=w_aug[0:NIN, WC + 0:WC + H_SP], in_=pw_hc[:, 1, :], mul=0.8)
    nc.scalar.mul(out=w_aug[0:NIN, WC + 8:WC + 8 + H_SP], in_=pw_hc[:, 1, :], mul=0.2)
    ai_hc = aintp[:].rearrange("f (h c) -> f c h", c=2)
    nc.scalar.mul(out=w_aug[NIN:NIN + D_INT, WC + H_SP:WC + 8], in_=ai_hc[:, 1, :], mul=0.8)
    nc.scalar.mul(out=w_aug[NIN:NIN + D_INT, WC + 8 + H_SP:WC + 16], in_=ai_hc[:, 1, :], mul=0.2)
    # wqa [97, 8]: +0.8 * a1 projections (D-row: exp(0.8 f1))
    wqa = big.tile([XR, NHEADS], F32, tag="wqa")
    nc.vector.memset(wqa, 0.0)
    nc.scalar.mul(out=wqa[0:NIN, 0:H_SP], in_=pw_hc[:, 0, :], mul=0.8)
    nc.scalar.mul(out=wqa[NIN:NIN + D_INT, H_SP:NHEADS], in_=ai_hc[:, 0, :], mul=0.8)

    # ---------------- Q rows + broadcast -----------------------------------
    pq = ps.tile([NHEADS, R], F32, tag="ps")
    nc.tensor.matmul(pq, wqa, xio)
    qrow = big.tile([NHEADS, R], BF16, tag="qrow")
    nc.scalar.activation(out=qrow, in_=pq, func=Act.Exp)
    qrd = dram.tile([NHEADS, R], BF16, tag="qrd")
    nc.gpsimd.dma_start(out=qrd, in_=qrow)
    qb = big.tile([128, NHEADS, R], BF16, tag="qb")
    for h in HEAD_ORDER:
        nc.gpsimd.dma_start(out=qb[:, h, :],
                            in_=qrd[h:h + 1, :].to_broadcast([128, R]))
    nc.gpsimd.dma_start(out=wout_f, in_=d_wout.ap().rearrange("(c p) o -> p c o", p=128))
    nc.gpsimd.dma_start(out=aout_sb, in_=d_aout.ap().rearrange("(c o) -> o c", c=2))
    for g in range(4):
        nc.gpsimd.dma_start(out=xia[:, 1024 * g:1024 * (g + 1)],
                            in_=d_xia.ap()[:, 1024 * g:1024 * (g + 1)])
    ones1 = big.tile([1, 128], F32, tag="ones1")
    nc.vector.memset(ones1, 1.0)
    neg1 = big.tile([128, 1], F32, tag="neg1")
    nc.vector.memset(neg1, -1.0)

    # ---------------- whp + P/C scores: built just-in-time in the jc loop --
    etP = big.tile([128, JT, 8], F32, tag="etP")
    whp = big.tile([128, JT, NHEADS, 33], BF16, tag="whp")

    # ---------------- layer 1 attention ------------------------------------
    # jc-outer: per key tile, all 8 heads' scores t8 (TS on DVE or relu on
    # ACT for assisted heads), one batched mask TT (adj broadcast across
    # heads) + gpsimd singles, then value-stationary MMs. ACT-assisted heads
    # compute t-1 via relu and add the +1 term with a second MM on raw adj.
    accs = [pacc.tile([97, R], F32, tag=f"acc{b}",
                      name=f"acc_l1_{b}") for b in range(NBANK)]
    PAT = os.environ.get("KERNEL_PAT", "AVAVAGAV")  # per (jc*8+h) pair
    GPH = int(os.environ.get("KERNEL_GPH", "2"))    # heads 8-GPH.. on gpsimd TT
    for jc in range(JT):
        pbld = psb.tile([128, WC + 16], F32, tag="pbld")
        nc.tensor.matmul(pbld, xia[:, 128 * jc:128 * (jc + 1)], w_aug)
        nc.scalar.activation(out=etP[:, jc, :], in_=pbld[:, WC:WC + 8],
                             func=Act.Exp)
        etC = ext.tile([128, 8], BF16, tag="etC")
        nc.scalar.activation(out=etC, in_=pbld[:, WC + 8:WC + 16], func=Act.Exp)
        nc.vector.tensor_tensor(
            whp[:, jc, :, :],
            pbld[:, 0:WC].rearrange("p (h c) -> p h c", c=33),
            etC[:].to_broadcast([128, 8, 33]), Alu.mult)
        t8 = work8.tile([128, NHEADS, R], BF16, tag="t8")
        kinds = []
        for h in range(NHEADS):
            kind = PAT[(jc * NHEADS + h) % len(PAT)]
            kinds.append(kind)
            if kind == "A":
                nc.scalar.activation(out=t8[:, h, :], in_=qb[:, h, :],
                                     func=Act.Relu,
                                     scale=etP[:, jc, h:h + 1], bias=neg1[:, 0:1])
            else:
                nc.vector.tensor_scalar(out=t8[:, h, :], in0=qb[:, h, :],
                                        scalar1=etP[:, jc, h:h + 1],
                                        scalar2=1.0, op0=Alu.mult, op1=Alu.max)
        m8 = work8.tile([128, NHEADS, R], BF16, tag="m8")
        nv = NHEADS - GPH
        aap = adjT_sb[:, jc, :]
        if nv > 0:
            bc = bass.AP(aap.tensor, aap.offset,
                         [list(aap.ap[0]), [0, nv], list(aap.ap[1])])
            nc.vector.tensor_tensor(m8[:, 0:nv, :], t8[:, 0:nv, :], bc, Alu.mult)
        for h in range(nv, NHEADS):
            nc.gpsimd.tensor_tensor(m8[:, h, :], t8[:, h, :], aap, Alu.mult)
        for h in range(NHEADS):
            b, slot = h // 2, h % 2
            out_sl = accs[b][64 * slot:64 * slot + 33, :]
            if kinds[h] == "A":
                nc.tensor.matmul(out_sl, whp[:, jc, h, :], aap,
                                 start=(jc == 0), stop=False)
                nc.tensor.matmul(out_sl, whp[:, jc, h, :], m8[:, h, :],
                                 start=False, stop=(jc == JT - 1))
            else:
                nc.tensor.matmul(out_sl, whp[:, jc, h, :], m8[:, h, :],
                                 start=(jc == 0), stop=(jc == JT - 1))

    # ---------------- l1 epilogue: transpose, normalize, elu ---------------
    accs_sb = []
    for b in range(NBANK):
        t_sb = big.tile([97, R], F32, tag=f"accsb{b}")
        nc.scalar.copy(out=t_sb, in_=accs[b])
        accs_sb.append(t_sb)
    h_nat = big.tile([128, IT, NHEADS * NHID], BF16, tag="h_nat")
    for it in range(IT):
        psq = ps.tile([128, NBANK * 97], F32, tag="ps")
        for b in range(NBANK):
            tr(psq[:, 97 * b:97 * (b + 1)],
               accs_sb[b][:, 128 * it:128 * (it + 1)], ident)
        psq_b = psq.rearrange("p (b c) -> p b c", c=97)
        rec = ext.tile([128, NHEADS], F32, tag="rec")
        rec_h = rec.rearrange("p (b s) -> p b s", s=2)
        nc.vector.reciprocal(out=rec_h[:, :, 0], in_=psq_b[:, :, 0])
        nc.vector.reciprocal(out=rec_h[:, :, 1], in_=psq_b[:, :, 64])
        v = ext.tile([128, NHEADS * NHID], BF16, tag="v")
        vv = v.rearrange("p (h o) -> p h o", h=NHEADS)
        for h in range(NHEADS):
            nc.vector.tensor_scalar(
                out=vv[:, h, :],
                in0=psq_b[:, h // 2, 64 * (h % 2) + 1:64 * (h % 2) + 33],
                scalar1=rec_h[:, h // 2, h % 2:h % 2 + 1], scalar2=None,
                op0=Alu.mult)
        e = ext.tile([128, NHEADS * NHID], BF16, tag="e")
        nc.scalar.activation(out=e, in_=v, func=Act.Exp)
        em1 = ext.tile([128, NHEADS * NHID], BF16, tag="em1")
        nc.vector.tensor_scalar(out=em1, in0=e, scalar1=-1.0, scalar2=None,
                                op0=Alu.add)
        r = ext.tile([128, NHEADS * NHID], BF16, tag="r")
        nc.vector.tensor_scalar(out=r, in0=v, scalar1=0.0, scalar2=None,
                                op0=Alu.max)
        nc.vector.tensor_tensor(h_nat[:, it, :], em1, r, Alu.min)

    # ---------------- output layer: Who, o1/o2, payload --------------------
    hT = big.tile([128, 2, R], BF16, tag="hT")
    for fc in range(2):
        ph = ps.tile([128, R], BF16, tag="ps")
        for it in range(IT):
            tr(ph[:, 128 * it:128 * (it + 1)],
               h_nat[:, it, 128 * fc:128 * (fc + 1)], ident_b)
        nc.scalar.copy(out=hT[:, fc, :], in_=ph)
    wout_m = big.tile([128, 2, NOUT], BF16, tag="wout_m")
    nc.scalar.copy(out=wout_m, in_=wout_f)
    pwho = ps.tile([NOUT, R], F32, tag="ps")
    for fc in range(2):
        nc.tensor.matmul(pwho, wout_m[:, fc, :], hT[:, fc, :],
                         start=(fc == 0), stop=(fc == 1))
    whoT_f = big.tile([NOUT, R], F32, tag="whoT_f")
    nc.scalar.copy(out=whoT_f, in_=pwho)
    whoT_b = big.tile([NOUT, R], BF16, tag="whoT_b")
    nc.scalar.copy(out=whoT_b, in_=pwho)
    po1 = ps.tile([1, R], F32, tag="ps")
    nc.tensor.matmul(po1, aout_sb[:, 0:1], whoT_f)
    po2 = ps.tile([1, R], F32, tag="ps")
    nc.tensor.matmul(po2, aout_sb[:, 1:2], whoT_f)
    # Qo row = exp(-0.8 o1) broadcast down partitions via K=1 matmul
    qo_row = big.tile([1, R], F32, tag="qo_row")
    nc.scalar.activation(out=qo_row, in_=po1, func=Act.Exp, scale=0.8)
    o2_row = big.tile([1, R], F32, tag="o2_row")
    nc.scalar.copy(out=o2_row, in_=po2)
    pqob = ps.tile([128, R], F32, tag="ps")
    nc.tensor.matmul(pqob, ones1, qo_row)
    qob = big.tile([128, R], BF16, tag="qob")
    nc.scalar.copy(out=qob, in_=pqob)
    # o2 transposed for payload P/C columns
    po2T = ps.tile([128, IT], F32, tag="ps")
    for it in range(IT):
        tr(po2T[:, it:it + 1], o2_row[:, 128 * it:128 * (it + 1)], ident)
    coT = big.tile([128, IT], F32, tag="coT")
    nc.scalar.activation(out=coT, in_=po2T, func=Act.Exp, scale=0.2)
    poT = big.tile([128, IT], F32, tag="poT")
    nc.scalar.activation(out=poT, in_=po2T, func=Act.Exp, scale=0.8)
    # payload [R, 66] built transposed: cols 0:64 Co*Who, 64 Co, 65 Po
    payT = big.tile([128, IT, NOUT + 2], BF16, tag="payT")
    for it in range(IT):
        ppt = ps.tile([128, NOUT], BF16, tag="ps")
        tr(ppt, whoT_b[:, 128 * it:128 * (it + 1)], ident_b)
        nc.scalar.mul(out=payT[:, it, 0:NOUT], in_=ppt, mul=coT[:, it:it + 1])
        nc.scalar.copy(out=payT[:, it, NOUT:NOUT + 1], in_=coT[:, it:it + 1])
        nc.scalar.copy(out=payT[:, it, NOUT + 1:NOUT + 2], in_=poT[:, it:it + 1])
    ccin = dram.tile([R, NOUT + 2], BF16, tag="ccin")
    ccout = dram.tile([N, NOUT + 2], BF16, tag="ccout")
    nc.sync.dma_start(out=ccin.rearrange("(k p) c -> p k c", p=128), in_=payT)
    if os.environ.get("KERNEL_SIMCC"):
        for d in range(NCORES):
            nc.sync.dma_start(out=ccout[R * d:R * (d + 1), :], in_=ccin)
    else:
        nc.gpsimd.collective_compute(
            "AllGather", mybir.AluOpType.bypass,
            replica_groups=[list(range(NCORES))],
            ins=[ccin.opt()], outs=[ccout.opt()])
    whop = big.tile([128, JT, NOUT + 2], BF16, tag="whop")
    nc.sync.dma_start(out=whop, in_=ccout.rearrange("(t p) c -> p t c", p=128))
    poT2 = big.tile([128, JT], F32, tag="poT2")
    nc.scalar.copy(out=poT2, in_=whop[:, :, NOUT + 1])

    # ---------------- output attention -------------------------------------
    acc2 = pacc.tile([NOUT + 1, R], F32, tag="acc0", name="acc_l2")
    PAT2 = os.environ.get("KERNEL_PAT2", "AVAV")
    for jg in range(JT // 4):
        t4 = work8.tile([128, 4, R], BF16, tag="t4")
        kinds2 = []
        for q in range(4):
            jc = 4 * jg + q
            kind = PAT2[jc % len(PAT2)]
            kinds2.append(kind)
            if kind == "A":
                nc.scalar.activation(out=t4[:, q, :], in_=qob,
                                     func=Act.Relu,
                                     scale=poT2[:, jc:jc + 1], bias=neg1[:, 0:1])
            else:
                nc.vector.tensor_scalar(out=t4[:, q, :], in0=qob,
                                        scalar1=poT2[:, jc:jc + 1],
                                        scalar2=1.0, op0=Alu.mult, op1=Alu.max)
        m4 = work8.tile([128, 4, R], BF16, tag="m4")
        nc.vector.tensor_tensor(m4, t4, adjT_sb[:, 4 * jg:4 * jg + 4, :],
                                Alu.mult)
        for q in range(4):
            jc = 4 * jg + q
            first = jc == 0
            last = jc == JT - 1
            if kinds2[q] == "A":
                nc.tensor.matmul(acc2, whop[:, jc, 0:NOUT + 1],
                                 adjT_sb[:, jc, :], start=first, stop=False)
                nc.tensor.matmul(acc2, whop[:, jc, 0:NOUT + 1], m4[:, q, :],
                                 start=False, stop=last)
            else:
                nc.tensor.matmul(acc2, whop[:, jc, 0:NOUT + 1], m4[:, q, :],
                                 start=first, stop=last)

    # ---------------- out = tanh(num/den) ----------------------------------
    acc2_sb = big.tile([NOUT + 1, R], F32, tag="acc2_sb")
    nc.scalar.copy(out=acc2_sb, in_=acc2)
    out_sb = big.tile([128, IT, NOUT], F32, tag="out_sb")
    for it in range(IT):
        psq2 = ps.tile([128, NOUT + 1], F32, tag="ps")
        tr(psq2, acc2_sb[:, 128 * it:128 * (it + 1)], ident)
        rec2 = ext.tile([128, 1], F32, tag="rec2")
        nc.vector.reciprocal(out=rec2, in_=psq2[:, NOUT:NOUT + 1])
        nc.scalar.activation(out=out_sb[:, it, :], in_=psq2[:, 0:NOUT],
                             func=Act.Tanh, scale=rec2)
    nc.sync.dma_start(out=d_out.ap().rearrange("(k p) c -> p k c", p=128),
                      in_=out_sb)
    ctx.close()


_NC_CACHE = None


def _get_nc():
    global _NC_CACHE
    if _NC_CACHE is None:
        _NC_CACHE = _build_program()
    return _NC_CACHE


def _make_in_maps(inputs):
    x = np.asarray(inputs["x"], np.float32)
    adj = np.asarray(inputs["adj"], np.float32)
    ie = np.asarray(inputs["intent_embeds"], np.float32)
    xia_full = np.concatenate(
        [x.T, ie.T, np.ones((1, N), np.float32)], axis=0)
    xia_full = np.ascontiguousarray(xia_full)
    in_maps = []
    for d in range(NCORES):
        sl = slice(d * R, (d + 1) * R)
        in_maps.append({
            "xia": xia_full,
            "xio": np.ascontiguousarray(xia_full[:, sl]),
            "adjT": np.ascontiguousarray(adj[sl, :].T).astype(ml_dtypes.bfloat16),
            "wsp": np.asarray(inputs["W_sp"], np.float32),
            "asp": np.asarray(inputs["a_sp"], np.float32),
            "wint": np.asarray(inputs["W_int"], np.float32),
            "aint": np.asarray(inputs["a_int"], np.float32),
            "wout": np.asarray(inputs["W_out"], np.float32),
            "aout": np.asarray(inputs["a_out"], np.float32),
        })
    return in_maps


def kernel(x, adj, intent_embeds, W_sp, a_sp, W_int, a_int, W_out, a_out):
    nc = _get_nc()
    in_maps = _make_in_maps(dict(
        x=x, adj=adj, intent_embeds=intent_embeds, W_sp=W_sp, a_sp=a_sp,
        W_int=W_int, a_int=a_int, W_out=W_out, a_out=a_out))
    res = run_bass_kernel_spmd(nc, in_maps, list(range(NCORES)))
    return np.concatenate([res.results[d]["out"] for d in range(NCORES)], axis=0)
